# revision 18
# baseline (speedup 1.0000x reference)
"""Bass/Trainium2 kernel for nn_DTree (soft decision tree MoE routing).

Contract: kernel(**inputs) takes the FULL unsharded inputs (np/jax arrays,
keyed as in setup_inputs()) and returns the FULL [8192, 256] float32 output.

Strategy: pure data parallelism over 8 NeuronCores — the batch (8192) is
sharded 1024 rows/core, all parameters are replicated, and the single
cross-batch quantity (mean entropy -> output scale) is combined with one
32-byte AllReduce that overlaps with the main leaf matmul.

All compute runs on-device. Host-side work is limited to layout prep
(transposes / dtype casts / building the 0/1 route matrix from route_idx,
route_side) and concatenating the 8 per-core output shards.
"""

import sys

for _p in ("/opt/trn_rl_repo",):
    if _p not in sys.path:
        sys.path.insert(0, _p)

import numpy as np
import ml_dtypes

import concourse.bass as bass
import concourse.mybir as mybir
import concourse.tile as tile
from concourse import bacc
from concourse.bass_utils import run_bass_kernel_spmd
from concourse.masks import make_identity

F32 = mybir.dt.float32
BF16 = mybir.dt.bfloat16
AF = mybir.ActivationFunctionType
ALU = mybir.AluOpType

N_CORES = 8
BATCH = 8192
B_CORE = BATCH // N_CORES  # 1024
D_IN = 512
D1 = 513
NUM_NODES = 127
NUM_LEAVES = 128
D_OUT = 256
DEPTH = 6
EPS = 0.01
MAX_ENT = (NUM_LEAVES / DEPTH) * float(np.log(DEPTH))

N_BT = B_CORE // 128          # 8 batch tiles per core
N_KT = D_IN // 128            # 4 contraction tiles (d = 0..511)
N_NC = (NUM_LEAVES * D_OUT) // 512  # 64 leaf column chunks of 512 (2 leaves)

_COMPILED = None  # (nc, input_names) memo


def _build_program(with_collective=True):
    nc = bacc.Bacc("TRN2", target_bir_lowering=False, debug=False,
                   num_devices=N_CORES if with_collective else 1)

    # ---- per-core DRAM I/O ----
    xT_c = nc.dram_tensor("xT_c", [D_IN, B_CORE], BF16, kind="ExternalInput")
    wpre = nc.dram_tensor("wpre", [D_IN, D1], BF16, kind="ExternalInput")
    bpre = nc.dram_tensor("bpre", [D1, 1], F32, kind="ExternalInput")
    nwt = nc.dram_tensor("nwt", [D1, NUM_NODES], BF16, kind="ExternalInput")
    mroute = nc.dram_tensor("mroute", [256, 128], F32, kind="ExternalInput")
    wleaf = nc.dram_tensor("wleaf", [D_IN, NUM_LEAVES * D_OUT], BF16,
                           kind="ExternalInput")
    wbcat = nc.dram_tensor("wbcat", [NUM_LEAVES, 2 * D_OUT], BF16,
                           kind="ExternalInput")
    out_c = nc.dram_tensor("out_c", [B_CORE, D_OUT], F32,
                           kind="ExternalOutput")

    with tile.TileContext(nc) as tc:
        from contextlib import ExitStack
        with ExitStack() as ctx:
            const = ctx.enter_context(tc.tile_pool(name="const", bufs=1))
            work = ctx.enter_context(tc.tile_pool(name="work", bufs=1))
            dram = ctx.enter_context(tc.tile_pool(name="dram", bufs=1,
                                                  space="DRAM"))

            # ---- constants into SBUF ----
            xT_sb = const.tile([128, N_KT * B_CORE], BF16, tag="xt")
            for kt in range(N_KT):
                nc.sync.dma_start(
                    xT_sb[:, kt * B_CORE:(kt + 1) * B_CORE],
                    xT_c[kt * 128:(kt + 1) * 128, :])

            wpre_sb = const.tile([128, N_KT * D1], BF16, tag="wpre")
            for kt in range(N_KT):
                nc.sync.dma_start(
                    wpre_sb[:, kt * D1:(kt + 1) * D1],
                    wpre[kt * 128:(kt + 1) * 128, :])

            bpre_sb = const.tile([128, 5], F32, tag="bpre")
            for mc in range(5):
                pc = 128 if mc < 4 else 1
                nc.sync.dma_start(bpre_sb[0:pc, mc:mc + 1],
                                  bpre[mc * 128:mc * 128 + pc, :])

            nwt_sb = const.tile([128, N_KT * NUM_NODES], BF16, tag="nwt")
            for kt in range(N_KT):
                nc.sync.dma_start(
                    nwt_sb[:, kt * NUM_NODES:(kt + 1) * NUM_NODES],
                    nwt[kt * 128:(kt + 1) * 128, :])
            nwt_last = const.tile([1, NUM_NODES], BF16, tag="nwtl")
            nc.sync.dma_start(nwt_last[0:1, :], nwt[512:513, :])

            m_sb = const.tile([128, 256], F32, tag="m")
            nc.sync.dma_start(m_sb[:, 0:128], mroute[0:128, :])
            nc.sync.dma_start(m_sb[:, 128:256], mroute[128:256, :])

            wb_sb = const.tile([128, 2 * D_OUT], BF16, tag="wb")
            nc.sync.dma_start(wb_sb[:, :], wbcat[:, :])

            ident = const.tile([128, 128], F32, tag="ident")
            make_identity(nc, ident[:, :])
            ones_col = const.tile([128, 1], F32, tag="onesc")
            nc.vector.memset(ones_col[:, :], 1.0)
            ones_row = const.tile([1, 128], F32, tag="onesr")
            nc.vector.memset(ones_row[:, :], 1.0)
            zero_col = const.tile([128, 1], F32, tag="zeroc")
            nc.vector.memset(zero_col[:, :], 0.0)
            half_col = const.tile([128, 1], F32, tag="halfc")
            nc.vector.memset(half_col[:, :], 0.5)
            ones_col_b = const.tile([128, 1], BF16, tag="onescb")
            nc.vector.memset(ones_col_b[:, :], 1.0)
            ones_row_b = const.tile([1, 128], BF16, tag="onesrb")
            nc.vector.memset(ones_row_b[:, :], 1.0)

            # ---- persistent intermediates ----
            hT = work.tile([128, N_KT * B_CORE], F32, tag="hT")
            h512 = work.tile([1, B_CORE], F32, tag="h512")
            h512b = work.tile([1, B_CORE], BF16, tag="h512b")
            hTb = work.tile([128, N_KT * B_CORE], BF16, tag="hTb")
            sq = work.tile([128, N_KT * B_CORE], BF16, tag="sq")
            sq_last = work.tile([1, B_CORE], BF16, tag="sql")
            rhh_row = work.tile([1, B_CORE], F32, tag="rhhr")
            rhh_row_b = work.tile([1, B_CORE], BF16, tag="rhhrb")
            rhh_bc = work.tile([128, B_CORE], F32, tag="rhhbc")
            cT = work.tile([128, B_CORE], F32, tag="cT")
            tA = work.tile([128, B_CORE], F32, tag="tA")
            tB = work.tile([128, B_CORE], F32, tag="tB")
            swT = work.tile([128, B_CORE], F32, tag="swT")
            swTb = work.tile([128, B_CORE], BF16, tag="swTb")
            entt = work.tile([128, B_CORE], F32, tag="entt")
            sw_all = work.tile([128, B_CORE], F32, tag="swall")
            h5c = work.tile([128, N_BT], F32, tag="h5c")
            srow = work.tile([1, 8], F32, tag="srow")
            stot = work.tile([1, 8], F32, tag="stot")
            scal = work.tile([1, 1], F32, tag="scal")
            scol = work.tile([128, 1], F32, tag="scol")
            acc = work.tile([128, N_BT * D_OUT], F32, tag="acc")

            NB2 = B_CORE // 512  # 2 chunks of 512 batch cols

            # ---- PSUM pools: 2 banks for phase A ("pa"), 6 for leaf ----
            papool = ctx.enter_context(
                tc.tile_pool(name="papool", bufs=2, space="PSUM"))
            ypool = ctx.enter_context(
                tc.tile_pool(name="ypool", bufs=6, space="PSUM"))
            wpool = ctx.enter_context(tc.tile_pool(name="wpool", bufs=8))
            opool = ctx.enter_context(tc.tile_pool(name="opool", bufs=2))
            tpool = ctx.enter_context(tc.tile_pool(name="tpool", bufs=6))

            # row 127 of tA/tB must be 0 (M rows 127/255 are zero)
            nc.vector.memset(tA[:, :], 0.0)
            nc.vector.memset(tB[:, :], 0.0)
            nc.vector.memset(srow[0:1, :], 0.0)

            # ======== phase A, two batch halves interleaved per step =====
            def stepA1(ns):
                nsl = slice(ns * 512, (ns + 1) * 512)
                for mc in range(5):
                    pc = 128 if mc < 4 else 1
                    ph = papool.tile([128, 512], F32, tag="pa")
                    for kt in range(N_KT):
                        nc.tensor.matmul(
                            ph[0:pc, :],
                            wpre_sb[:, kt * D1 + mc * 128:
                                    kt * D1 + mc * 128 + pc],
                            xT_sb[:, kt * B_CORE + ns * 512:
                                  kt * B_CORE + (ns + 1) * 512],
                            start=(kt == 0), stop=(kt == N_KT - 1))
                    if mc < 4:
                        csl = slice(mc * B_CORE + ns * 512,
                                    mc * B_CORE + (ns + 1) * 512)
                        nc.scalar.activation(hT[:, csl], ph[0:pc, :], AF.Relu,
                                             bias=bpre_sb[0:pc, mc:mc + 1])
                        # bf16 relu directly from psum on DVE (parallel w/ ACT)
                        nc.vector.tensor_scalar(hTb[:, csl], ph[0:pc, :],
                                                bpre_sb[0:pc, mc:mc + 1], 0.0,
                                                op0=ALU.add, op1=ALU.max)
                    else:
                        nc.scalar.activation(h512[0:1, nsl], ph[0:pc, :],
                                             AF.Relu,
                                             bias=bpre_sb[0:pc, mc:mc + 1])
                        nc.vector.tensor_scalar(h512b[0:1, nsl], ph[0:pc, :],
                                                bpre_sb[0:pc, mc:mc + 1], 0.0,
                                                op0=ALU.add, op1=ALU.max)

            def stepSQ(ns):
                nsl = slice(ns * 512, (ns + 1) * 512)
                for mc in range(4):
                    csl = slice(mc * B_CORE + ns * 512,
                                mc * B_CORE + (ns + 1) * 512)
                    if mc < 2:
                        nc.vector.tensor_tensor(sq[:, csl], hT[:, csl],
                                                hT[:, csl], op=ALU.mult)
                    else:
                        nc.scalar.activation(sq[:, csl], hT[:, csl],
                                             AF.Square, bias=zero_col[:, 0:1])
                nc.scalar.activation(sq_last[0:1, nsl], h512[0:1, nsl],
                                     AF.Square, bias=zero_col[0:1, 0:1])

            def stepHH(ns):
                nsl = slice(ns * 512, (ns + 1) * 512)
                phh = papool.tile([1, 512], F32, tag="pa")
                for mc in range(5):
                    pc = 128 if mc < 4 else 1
                    rhs = (sq[:, mc * B_CORE + ns * 512:
                              mc * B_CORE + (ns + 1) * 512]
                           if mc < 4 else sq_last[0:1, nsl])
                    nc.tensor.matmul(phh[0:1, :], ones_col_b[0:pc, 0:1], rhs,
                                     start=(mc == 0), stop=(mc == 4))
                nc.scalar.activation(rhh_row[0:1, nsl], phh[0:1, :], AF.Sqrt,
                                     bias=zero_col[0:1, 0:1])
                nc.vector.tensor_scalar_max(rhh_row[0:1, nsl],
                                            rhh_row[0:1, nsl], 1e-12)
                nc.vector.reciprocal(rhh_row[0:1, nsl], rhh_row[0:1, nsl])
                nc.scalar.activation(rhh_row_b[0:1, nsl], rhh_row[0:1, nsl],
                                     AF.Copy)

            def stepRIGHT(ns):
                nsl = slice(ns * 512, (ns + 1) * 512)
                pbc = papool.tile([128, 512], F32, tag="pa")
                nc.tensor.matmul(pbc[:, :], ones_row_b[0:1, 0:128],
                                 rhh_row_b[0:1, nsl], start=True, stop=True)
                nc.scalar.activation(rhh_bc[:, nsl], pbc[:, :], AF.Copy)
                prT = papool.tile([128, 512], F32, tag="pa")
                for kt in range(5):
                    pc = 128 if kt < 4 else 1
                    lhsT = (nwt_sb[:, kt * NUM_NODES:(kt + 1) * NUM_NODES]
                            if kt < 4 else nwt_last[0:1, :])
                    rhs = (hTb[:, kt * B_CORE + ns * 512:
                               kt * B_CORE + (ns + 1) * 512]
                           if kt < 4 else h512b[0:1, nsl])
                    nc.tensor.matmul(prT[0:NUM_NODES, :], lhsT, rhs,
                                     start=(kt == 0), stop=(kt == 4))
                nc.vector.tensor_tensor(cT[0:NUM_NODES, nsl],
                                        prT[0:NUM_NODES, :],
                                        rhh_bc[0:NUM_NODES, nsl], op=ALU.mult)
                # clamp cosine to [-0.98, 0.98] (== prob clip [0.01, 0.99])
                nc.vector.tensor_scalar(cT[0:NUM_NODES, nsl],
                                        cT[0:NUM_NODES, nsl],
                                        0.98, -0.98, op0=ALU.min, op1=ALU.max)

            def stepLOGP(ns):
                nsl = slice(ns * 512, (ns + 1) * 512)
                nc.scalar.activation(tA[0:NUM_NODES, nsl],
                                     cT[0:NUM_NODES, nsl], AF.Ln, scale=-0.5,
                                     bias=half_col[0:NUM_NODES, 0:1])
                nc.scalar.activation(tB[0:NUM_NODES, nsl],
                                     cT[0:NUM_NODES, nsl], AF.Ln, scale=0.5,
                                     bias=half_col[0:NUM_NODES, 0:1])
                plp = papool.tile([128, 512], F32, tag="pa")
                nc.tensor.matmul(plp[:, :], m_sb[:, 0:128], tA[:, nsl],
                                 start=True, stop=False)
                nc.tensor.matmul(plp[:, :], m_sb[:, 128:256], tB[:, nsl],
                                 start=False, stop=True)
                nc.scalar.activation(swT[:, nsl], plp[:, :], AF.Exp,
                                     bias=zero_col[:, 0:1])
                nc.vector.tensor_copy(swTb[:, nsl], swT[:, nsl])
                nc.vector.tensor_tensor(entt[:, nsl], plp[:, :], swT[:, nsl],
                                        op=ALU.mult)
                pent = papool.tile([1, 512], F32, tag="pa")
                nc.tensor.matmul(pent[0:1, :], ones_col[:, 0:1], entt[:, nsl],
                                 start=True, stop=True)
                nc.vector.reduce_sum(srow[0:1, ns:ns + 1], pent[0:1, :],
                                     axis=mybir.AxisListType.X)

            def stepTRANS(ns):
                for bt in range(4 * ns, 4 * ns + 4):
                    sl = slice(bt * 128, (bt + 1) * 128)
                    pt = papool.tile([128, 128], F32, tag="pa")
                    nc.tensor.matmul(pt[:, :], swT[:, sl], ident[:, :],
                                     start=True, stop=True)
                    nc.scalar.activation(sw_all[:, sl], pt[:, :], AF.Copy)
                    pt5 = papool.tile([128, 1], F32, tag="pa")
                    nc.tensor.matmul(pt5[:, 0:1], h512[0:1, sl],
                                     ones_row[0:1, 0:1],
                                     start=True, stop=True)
                    nc.scalar.activation(h5c[:, bt:bt + 1], pt5[:, 0:1],
                                         AF.Copy)
                # init acc for this half: acc = C2 + h512 * C1
                for bt in range(4 * ns, 4 * ns + 4):
                    pc12 = ypool.tile([128, 512], F32, tag="y")
                    nc.tensor.matmul(pc12[:, :],
                                     swTb[:, bt * 128:(bt + 1) * 128],
                                     wb_sb[:, :], start=True, stop=True)
                    asl = acc[:, bt * D_OUT:(bt + 1) * D_OUT]
                    nc.scalar.activation(asl, pc12[:, D_OUT:2 * D_OUT],
                                         AF.Copy)
                    nc.vector.scalar_tensor_tensor(
                        asl, pc12[:, 0:D_OUT], h5c[:, bt:bt + 1], asl,
                        op0=ALU.mult, op1=ALU.add)

            for step in (stepA1, stepSQ, stepHH, stepRIGHT, stepLOGP,
                         stepTRANS):
                for ns in range(NB2):
                    step(ns)

            # ======== allreduce of entropy partials -> output scale ======
            ccin = dram.tile([1, 8], F32)
            ccout = dram.tile([1, 8], F32)
            nc.sync.dma_start(ccin[:], srow[0:1, :])
            if with_collective:
                nc.gpsimd.collective_compute(
                    "AllReduce", ALU.add,
                    replica_groups=[list(range(N_CORES))],
                    ins=[ccin.opt()], outs=[ccout.opt()])
                nc.sync.dma_start(stot[0:1, :], ccout[:])
            else:
                # single-core sim variant: no collective
                nc.sync.dma_start(stot[0:1, :], ccin[:])
            # scale = 1 - (S0+S1) / (BATCH * MAX_ENT), broadcast to [128,1]
            nc.vector.reduce_sum(scal[0:1, 0:1], stot[0:1, 0:2],
                                 axis=mybir.AxisListType.X)
            nc.vector.tensor_scalar(scal[0:1, 0:1], scal[0:1, 0:1],
                                    -1.0 / (BATCH * MAX_ENT), 1.0,
                                    op0=ALU.mult, op1=ALU.add)
            psc = papool.tile([128, 1], F32, tag="pa")
            nc.tensor.matmul(psc[:, 0:1], ones_row[0:1, :], scal[0:1, 0:1],
                             start=True, stop=True)
            nc.scalar.activation(scol[:, 0:1], psc[:, 0:1], AF.Copy)

            # ======== phase B: leaf matmul + weighted combine ============
            for ncx in range(N_NC):
                wts = []
                for kt in range(N_KT):
                    wt = wpool.tile([128, 512], BF16, tag=f"w{kt}")
                    nc.sync.dma_start(
                        wt[:, :],
                        wleaf[kt * 128:(kt + 1) * 128,
                              ncx * 512:(ncx + 1) * 512])
                    wts.append(wt)
                for bt in range(N_BT):
                    py = ypool.tile([128, 512], F32, tag="y")
                    for kt in range(N_KT):
                        nc.tensor.matmul(
                            py[:, :],
                            hTb[:, kt * B_CORE + bt * 128:
                                kt * B_CORE + (bt + 1) * 128],
                            wts[kt][:, :],
                            start=(kt == 0), stop=(kt == N_KT - 1))
                    l0 = 2 * ncx
                    asl = acc[:, bt * D_OUT:(bt + 1) * D_OUT]
                    nc.vector.scalar_tensor_tensor(
                        asl, py[:, 0:D_OUT],
                        sw_all[:, bt * 128 + l0:bt * 128 + l0 + 1],
                        asl, op0=ALU.mult, op1=ALU.add)
                    tmp = tpool.tile([128, D_OUT], F32, tag="tmp")
                    nc.scalar.activation(
                        tmp[:, :], py[:, D_OUT:2 * D_OUT], AF.Copy,
                        scale=sw_all[:, bt * 128 + l0 + 1:
                                     bt * 128 + l0 + 2])
                    nc.gpsimd.tensor_tensor(asl, tmp[:, :], asl,
                                            op=ALU.add)

            # ======== final scale and store ==============================
            for bt in range(N_BT):
                ot = opool.tile([128, D_OUT], F32, tag="o")
                nc.scalar.activation(
                    ot[:, :], acc[:, bt * D_OUT:(bt + 1) * D_OUT],
                    AF.Copy, scale=scol[:, 0:1])
                nc.sync.dma_start(out_c[bt * 128:(bt + 1) * 128, :],
                                  ot[:, :])

    nc.compile()
    return nc


def _prep_inputs(x, W_pre, b_pre, right_w, W_leaf, b_leaf, route_idx,
                 route_side):
    x = np.asarray(x, np.float32)
    W_pre = np.asarray(W_pre, np.float32)
    b_pre = np.asarray(b_pre, np.float32)
    right_w = np.asarray(right_w, np.float32)
    W_leaf = np.asarray(W_leaf, np.float32)
    b_leaf = np.asarray(b_leaf, np.float32)
    route_idx = np.asarray(route_idx)
    route_side = np.asarray(route_side)

    xT = np.ascontiguousarray(x.T).astype(ml_dtypes.bfloat16)  # [512, 8192]
    wpre = np.ascontiguousarray(W_pre.T).astype(ml_dtypes.bfloat16)
    bpre = np.ascontiguousarray(b_pre.reshape(D1, 1))
    nw = right_w / np.maximum(
        np.linalg.norm(right_w, axis=1, keepdims=True), 1e-12)
    nwt = np.ascontiguousarray(nw.T).astype(ml_dtypes.bfloat16)  # [513, 127]

    M = np.zeros((256, 128), np.float32)
    n_steps = route_idx.shape[1]
    for leaf in range(NUM_LEAVES):
        for d in range(n_steps):
            node = int(route_idx[leaf, d])
            side = int(route_side[leaf, d])
            M[node + (128 if side else 0), leaf] += 1.0

    wleaf = np.ascontiguousarray(W_leaf[:, :D_IN].T).astype(
        ml_dtypes.bfloat16)                             # [512, 32768]
    wbcat = np.ascontiguousarray(np.concatenate(
        [W_leaf[:, D_IN].reshape(NUM_LEAVES, D_OUT),
         b_leaf.reshape(NUM_LEAVES, D_OUT)],
        axis=1)).astype(ml_dtypes.bfloat16)             # [128, 512]

    shared = {"wpre": wpre, "bpre": bpre, "nwt": nwt, "mroute": M,
              "wleaf": wleaf, "wbcat": wbcat}
    in_maps = []
    for c in range(N_CORES):
        m = dict(shared)
        m["xT_c"] = np.ascontiguousarray(
            xT[:, c * B_CORE:(c + 1) * B_CORE])
        in_maps.append(m)
    return in_maps


def kernel(x, W_pre, b_pre, right_w, W_leaf, b_leaf, route_idx, route_side):
    global _COMPILED
    if _COMPILED is None:
        _COMPILED = _build_program()
    nc = _COMPILED
    in_maps = _prep_inputs(x, W_pre, b_pre, right_w, W_leaf, b_leaf,
                           route_idx, route_side)
    res = run_bass_kernel_spmd(nc, in_maps, core_ids=list(range(N_CORES)))
    out = np.concatenate([res.results[c]["out_c"] for c in range(N_CORES)],
                         axis=0)
    return out.astype(np.float32)


# revision 22
# speedup vs baseline: 1.0001x; 1.0001x over previous
"""Bass/Trainium2 kernel for nn_DTree (soft decision tree MoE routing).

Contract: kernel(**inputs) takes the FULL unsharded inputs (np/jax arrays,
keyed as in setup_inputs()) and returns the FULL [8192, 256] float32 output.

Strategy: pure data parallelism over 8 NeuronCores — the batch (8192) is
sharded 1024 rows/core, all parameters are replicated, and the single
cross-batch quantity (mean entropy -> output scale) is combined with one
32-byte AllReduce that overlaps with the main leaf matmul.

All compute runs on-device. Host-side work is limited to layout prep
(transposes / dtype casts / building the 0/1 route matrix from route_idx,
route_side) and concatenating the 8 per-core output shards.
"""

import sys

for _p in ("/opt/trn_rl_repo",):
    if _p not in sys.path:
        sys.path.insert(0, _p)

import numpy as np
import ml_dtypes

import concourse.bass as bass
import concourse.mybir as mybir
import concourse.tile as tile
from concourse import bacc
from concourse.bass_utils import run_bass_kernel_spmd
from concourse.masks import make_identity

F32 = mybir.dt.float32
BF16 = mybir.dt.bfloat16
AF = mybir.ActivationFunctionType
ALU = mybir.AluOpType

N_CORES = 8
BATCH = 8192
B_CORE = BATCH // N_CORES  # 1024
D_IN = 512
D1 = 513
NUM_NODES = 127
NUM_LEAVES = 128
D_OUT = 256
DEPTH = 6
EPS = 0.01
MAX_ENT = (NUM_LEAVES / DEPTH) * float(np.log(DEPTH))

N_BT = B_CORE // 128          # 8 batch tiles per core
N_KT = D_IN // 128            # 4 contraction tiles (d = 0..511)
N_NC = (NUM_LEAVES * D_OUT) // 512  # 64 leaf column chunks of 512 (2 leaves)

_COMPILED = None  # (nc, input_names) memo


def _build_program(with_collective=True):
    nc = bacc.Bacc("TRN2", target_bir_lowering=False, debug=False,
                   num_devices=N_CORES if with_collective else 1)

    # ---- per-core DRAM I/O ----
    xT_c = nc.dram_tensor("xT_c", [D_IN, B_CORE], BF16, kind="ExternalInput")
    wpre = nc.dram_tensor("wpre", [D_IN, D1], BF16, kind="ExternalInput")
    bpre = nc.dram_tensor("bpre", [D1, 1], F32, kind="ExternalInput")
    nwt = nc.dram_tensor("nwt", [D1, NUM_NODES], BF16, kind="ExternalInput")
    mroute = nc.dram_tensor("mroute", [256, 128], F32, kind="ExternalInput")
    wleaf = nc.dram_tensor("wleaf", [D_IN, NUM_LEAVES * D_OUT], BF16,
                           kind="ExternalInput")
    wbcat = nc.dram_tensor("wbcat", [NUM_LEAVES, 2 * D_OUT], BF16,
                           kind="ExternalInput")
    out_c = nc.dram_tensor("out_c", [B_CORE, D_OUT], F32,
                           kind="ExternalOutput")

    with tile.TileContext(nc) as tc:
        from contextlib import ExitStack
        with ExitStack() as ctx:
            const = ctx.enter_context(tc.tile_pool(name="const", bufs=1))
            work = ctx.enter_context(tc.tile_pool(name="work", bufs=1))
            dram = ctx.enter_context(tc.tile_pool(name="dram", bufs=1,
                                                  space="DRAM"))

            # ---- constants into SBUF ----
            xT_sb = const.tile([128, N_KT * B_CORE], BF16, tag="xt")
            for kt in range(N_KT):
                nc.sync.dma_start(
                    xT_sb[:, kt * B_CORE:(kt + 1) * B_CORE],
                    xT_c[kt * 128:(kt + 1) * 128, :])

            wpre_sb = const.tile([128, N_KT * D1], BF16, tag="wpre")
            for kt in range(N_KT):
                nc.sync.dma_start(
                    wpre_sb[:, kt * D1:(kt + 1) * D1],
                    wpre[kt * 128:(kt + 1) * 128, :])

            bpre_sb = const.tile([128, 5], F32, tag="bpre")
            for mc in range(5):
                pc = 128 if mc < 4 else 1
                nc.sync.dma_start(bpre_sb[0:pc, mc:mc + 1],
                                  bpre[mc * 128:mc * 128 + pc, :])

            nwt_sb = const.tile([128, N_KT * NUM_NODES], BF16, tag="nwt")
            for kt in range(N_KT):
                nc.sync.dma_start(
                    nwt_sb[:, kt * NUM_NODES:(kt + 1) * NUM_NODES],
                    nwt[kt * 128:(kt + 1) * 128, :])
            nwt_last = const.tile([1, NUM_NODES], BF16, tag="nwtl")
            nc.sync.dma_start(nwt_last[0:1, :], nwt[512:513, :])

            m_sb = const.tile([128, 256], F32, tag="m")
            nc.sync.dma_start(m_sb[:, 0:128], mroute[0:128, :])
            nc.sync.dma_start(m_sb[:, 128:256], mroute[128:256, :])

            wb_sb = const.tile([128, 2 * D_OUT], BF16, tag="wb")
            nc.sync.dma_start(wb_sb[:, :], wbcat[:, :])

            ident = const.tile([128, 128], F32, tag="ident")
            make_identity(nc, ident[:, :])
            ones_col = const.tile([128, 1], F32, tag="onesc")
            nc.vector.memset(ones_col[:, :], 1.0)
            ones_row = const.tile([1, 128], F32, tag="onesr")
            nc.vector.memset(ones_row[:, :], 1.0)
            zero_col = const.tile([128, 1], F32, tag="zeroc")
            nc.vector.memset(zero_col[:, :], 0.0)
            half_col = const.tile([128, 1], F32, tag="halfc")
            nc.vector.memset(half_col[:, :], 0.5)
            ones_col_b = const.tile([128, 1], BF16, tag="onescb")
            nc.vector.memset(ones_col_b[:, :], 1.0)
            ones_row_b = const.tile([1, 128], BF16, tag="onesrb")
            nc.vector.memset(ones_row_b[:, :], 1.0)

            # ---- persistent intermediates ----
            hT = work.tile([128, N_KT * B_CORE], F32, tag="hT")
            h512 = work.tile([1, B_CORE], F32, tag="h512")
            h512b = work.tile([1, B_CORE], BF16, tag="h512b")
            hTb = work.tile([128, N_KT * B_CORE], BF16, tag="hTb")
            sq = work.tile([128, N_KT * B_CORE], BF16, tag="sq")
            sq_last = work.tile([1, B_CORE], BF16, tag="sql")
            rhh_row = work.tile([1, B_CORE], F32, tag="rhhr")
            rhh_row_b = work.tile([1, B_CORE], BF16, tag="rhhrb")
            rhh_bc = work.tile([128, B_CORE], F32, tag="rhhbc")
            cT = work.tile([128, B_CORE], F32, tag="cT")
            tA = work.tile([128, B_CORE], F32, tag="tA")
            tB = work.tile([128, B_CORE], F32, tag="tB")
            swT = work.tile([128, B_CORE], F32, tag="swT")
            swTb = work.tile([128, B_CORE], BF16, tag="swTb")
            entt = work.tile([128, B_CORE], F32, tag="entt")
            sw_all = work.tile([128, B_CORE], F32, tag="swall")
            h5c = work.tile([128, N_BT], F32, tag="h5c")
            srow = work.tile([1, 8], F32, tag="srow")
            stot = work.tile([1, 8], F32, tag="stot")
            scal = work.tile([1, 1], F32, tag="scal")
            scol = work.tile([128, 1], F32, tag="scol")
            acc = work.tile([128, N_BT * D_OUT], F32, tag="acc")

            NB2 = B_CORE // 512  # 2 chunks of 512 batch cols

            # ---- PSUM pools: 2 banks for phase A ("pa"), 6 for leaf ----
            papool = ctx.enter_context(
                tc.tile_pool(name="papool", bufs=3, space="PSUM"))
            ypool = ctx.enter_context(
                tc.tile_pool(name="ypool", bufs=5, space="PSUM"))
            wpool = ctx.enter_context(tc.tile_pool(name="wpool", bufs=8))
            opool = ctx.enter_context(tc.tile_pool(name="opool", bufs=2))
            tpool = ctx.enter_context(tc.tile_pool(name="tpool", bufs=6))

            # row 127 of tA/tB must be 0 (M rows 127/255 are zero)
            nc.vector.memset(tA[:, :], 0.0)
            nc.vector.memset(tB[:, :], 0.0)
            nc.vector.memset(srow[0:1, :], 0.0)

            # ======== phase A, two batch halves interleaved per step =====
            def stepA1(ns):
                nsl = slice(ns * 512, (ns + 1) * 512)
                for mc in range(5):
                    pc = 128 if mc < 4 else 1
                    ph = papool.tile([128, 512], F32, tag="pa")
                    for kt in range(N_KT):
                        nc.tensor.matmul(
                            ph[0:pc, :],
                            wpre_sb[:, kt * D1 + mc * 128:
                                    kt * D1 + mc * 128 + pc],
                            xT_sb[:, kt * B_CORE + ns * 512:
                                  kt * B_CORE + (ns + 1) * 512],
                            start=(kt == 0), stop=(kt == N_KT - 1))
                    if mc < 4:
                        csl = slice(mc * B_CORE + ns * 512,
                                    mc * B_CORE + (ns + 1) * 512)
                        nc.scalar.activation(hT[:, csl], ph[0:pc, :], AF.Relu,
                                             bias=bpre_sb[0:pc, mc:mc + 1])
                        # bf16 relu directly from psum on DVE (parallel w/ ACT)
                        nc.vector.tensor_scalar(hTb[:, csl], ph[0:pc, :],
                                                bpre_sb[0:pc, mc:mc + 1], 0.0,
                                                op0=ALU.add, op1=ALU.max)
                    else:
                        nc.scalar.activation(h512[0:1, nsl], ph[0:pc, :],
                                             AF.Relu,
                                             bias=bpre_sb[0:pc, mc:mc + 1])
                        nc.vector.tensor_scalar(h512b[0:1, nsl], ph[0:pc, :],
                                                bpre_sb[0:pc, mc:mc + 1], 0.0,
                                                op0=ALU.add, op1=ALU.max)

            def stepSQ(ns):
                nsl = slice(ns * 512, (ns + 1) * 512)
                for mc in range(4):
                    csl = slice(mc * B_CORE + ns * 512,
                                mc * B_CORE + (ns + 1) * 512)
                    if mc < 2:
                        nc.vector.tensor_tensor(sq[:, csl], hT[:, csl],
                                                hT[:, csl], op=ALU.mult)
                    else:
                        nc.scalar.activation(sq[:, csl], hT[:, csl],
                                             AF.Square, bias=zero_col[:, 0:1])
                nc.scalar.activation(sq_last[0:1, nsl], h512[0:1, nsl],
                                     AF.Square, bias=zero_col[0:1, 0:1])

            def stepHH(ns):
                nsl = slice(ns * 512, (ns + 1) * 512)
                phh = papool.tile([1, 512], F32, tag="pa")
                for mc in range(5):
                    pc = 128 if mc < 4 else 1
                    rhs = (sq[:, mc * B_CORE + ns * 512:
                              mc * B_CORE + (ns + 1) * 512]
                           if mc < 4 else sq_last[0:1, nsl])
                    nc.tensor.matmul(phh[0:1, :], ones_col_b[0:pc, 0:1], rhs,
                                     start=(mc == 0), stop=(mc == 4))
                nc.scalar.activation(rhh_row[0:1, nsl], phh[0:1, :], AF.Sqrt,
                                     bias=zero_col[0:1, 0:1])
                nc.vector.tensor_scalar_max(rhh_row[0:1, nsl],
                                            rhh_row[0:1, nsl], 1e-12)
                nc.vector.reciprocal(rhh_row[0:1, nsl], rhh_row[0:1, nsl])
                nc.scalar.activation(rhh_row_b[0:1, nsl], rhh_row[0:1, nsl],
                                     AF.Copy)

            def stepRIGHT(ns):
                nsl = slice(ns * 512, (ns + 1) * 512)
                pbc = papool.tile([128, 512], F32, tag="pa")
                nc.tensor.matmul(pbc[:, :], ones_row_b[0:1, 0:128],
                                 rhh_row_b[0:1, nsl], start=True, stop=True)
                nc.scalar.activation(rhh_bc[:, nsl], pbc[:, :], AF.Copy)
                prT = papool.tile([128, 512], F32, tag="pa")
                for kt in range(5):
                    pc = 128 if kt < 4 else 1
                    lhsT = (nwt_sb[:, kt * NUM_NODES:(kt + 1) * NUM_NODES]
                            if kt < 4 else nwt_last[0:1, :])
                    rhs = (hTb[:, kt * B_CORE + ns * 512:
                               kt * B_CORE + (ns + 1) * 512]
                           if kt < 4 else h512b[0:1, nsl])
                    nc.tensor.matmul(prT[0:NUM_NODES, :], lhsT, rhs,
                                     start=(kt == 0), stop=(kt == 4))
                nc.vector.tensor_tensor(cT[0:NUM_NODES, nsl],
                                        prT[0:NUM_NODES, :],
                                        rhh_bc[0:NUM_NODES, nsl], op=ALU.mult)
                # clamp cosine to [-0.98, 0.98] (== prob clip [0.01, 0.99])
                nc.vector.tensor_scalar(cT[0:NUM_NODES, nsl],
                                        cT[0:NUM_NODES, nsl],
                                        0.98, -0.98, op0=ALU.min, op1=ALU.max)

            def stepLOGP(ns):
                nsl = slice(ns * 512, (ns + 1) * 512)
                nc.scalar.activation(tA[0:NUM_NODES, nsl],
                                     cT[0:NUM_NODES, nsl], AF.Ln, scale=-0.5,
                                     bias=half_col[0:NUM_NODES, 0:1])
                nc.scalar.activation(tB[0:NUM_NODES, nsl],
                                     cT[0:NUM_NODES, nsl], AF.Ln, scale=0.5,
                                     bias=half_col[0:NUM_NODES, 0:1])
                plp = papool.tile([128, 512], F32, tag="pa")
                nc.tensor.matmul(plp[:, :], m_sb[:, 0:128], tA[:, nsl],
                                 start=True, stop=False)
                nc.tensor.matmul(plp[:, :], m_sb[:, 128:256], tB[:, nsl],
                                 start=False, stop=True)
                nc.scalar.activation(swT[:, nsl], plp[:, :], AF.Exp,
                                     bias=zero_col[:, 0:1])
                nc.vector.tensor_copy(swTb[:, nsl], swT[:, nsl])
                nc.vector.tensor_tensor(entt[:, nsl], plp[:, :], swT[:, nsl],
                                        op=ALU.mult)
                pent = papool.tile([1, 512], F32, tag="pa")
                nc.tensor.matmul(pent[0:1, :], ones_col[:, 0:1], entt[:, nsl],
                                 start=True, stop=True)
                nc.vector.reduce_sum(srow[0:1, ns:ns + 1], pent[0:1, :],
                                     axis=mybir.AxisListType.X)

            def stepTRANS(ns):
                for bt in range(4 * ns, 4 * ns + 4):
                    sl = slice(bt * 128, (bt + 1) * 128)
                    pt = papool.tile([128, 128], F32, tag="pa")
                    nc.tensor.matmul(pt[:, :], swT[:, sl], ident[:, :],
                                     start=True, stop=True)
                    nc.scalar.activation(sw_all[:, sl], pt[:, :], AF.Copy)
                    pt5 = papool.tile([128, 1], F32, tag="pa")
                    nc.tensor.matmul(pt5[:, 0:1], h512[0:1, sl],
                                     ones_row[0:1, 0:1],
                                     start=True, stop=True)
                    nc.scalar.activation(h5c[:, bt:bt + 1], pt5[:, 0:1],
                                         AF.Copy)
                # init acc for this half: acc = C2 + h512 * C1
                for bt in range(4 * ns, 4 * ns + 4):
                    pc12 = ypool.tile([128, 512], F32, tag="y")
                    nc.tensor.matmul(pc12[:, :],
                                     swTb[:, bt * 128:(bt + 1) * 128],
                                     wb_sb[:, :], start=True, stop=True)
                    asl = acc[:, bt * D_OUT:(bt + 1) * D_OUT]
                    nc.scalar.activation(asl, pc12[:, D_OUT:2 * D_OUT],
                                         AF.Copy)
                    nc.vector.scalar_tensor_tensor(
                        asl, pc12[:, 0:D_OUT], h5c[:, bt:bt + 1], asl,
                        op0=ALU.mult, op1=ALU.add)

            for step in (stepA1, stepSQ, stepHH, stepRIGHT, stepLOGP,
                         stepTRANS):
                for ns in range(NB2):
                    step(ns)

            # ======== allreduce of entropy partials -> output scale ======
            ccin = dram.tile([1, 8], F32)
            ccout = dram.tile([1, 8], F32)
            nc.sync.dma_start(ccin[:], srow[0:1, :])
            if with_collective:
                nc.gpsimd.collective_compute(
                    "AllReduce", ALU.add,
                    replica_groups=[list(range(N_CORES))],
                    ins=[ccin.opt()], outs=[ccout.opt()])
                nc.sync.dma_start(stot[0:1, :], ccout[:])
            else:
                # single-core sim variant: no collective
                nc.sync.dma_start(stot[0:1, :], ccin[:])
            # scale = 1 - (S0+S1) / (BATCH * MAX_ENT), broadcast to [128,1]
            nc.vector.reduce_sum(scal[0:1, 0:1], stot[0:1, 0:2],
                                 axis=mybir.AxisListType.X)
            nc.vector.tensor_scalar(scal[0:1, 0:1], scal[0:1, 0:1],
                                    -1.0 / (BATCH * MAX_ENT), 1.0,
                                    op0=ALU.mult, op1=ALU.add)
            psc = papool.tile([128, 1], F32, tag="pa")
            nc.tensor.matmul(psc[:, 0:1], ones_row[0:1, :], scal[0:1, 0:1],
                             start=True, stop=True)
            nc.scalar.activation(scol[:, 0:1], psc[:, 0:1], AF.Copy)

            # ======== phase B: leaf matmul + weighted combine ============
            for ncx in range(N_NC):
                wts = []
                for kt in range(N_KT):
                    wt = wpool.tile([128, 512], BF16, tag=f"w{kt}")
                    nc.sync.dma_start(
                        wt[:, :],
                        wleaf[kt * 128:(kt + 1) * 128,
                              ncx * 512:(ncx + 1) * 512])
                    wts.append(wt)
                for bt in range(N_BT):
                    py = ypool.tile([128, 512], F32, tag="y")
                    for kt in range(N_KT):
                        nc.tensor.matmul(
                            py[:, :],
                            hTb[:, kt * B_CORE + bt * 128:
                                kt * B_CORE + (bt + 1) * 128],
                            wts[kt][:, :],
                            start=(kt == 0), stop=(kt == N_KT - 1))
                    l0 = 2 * ncx
                    asl = acc[:, bt * D_OUT:(bt + 1) * D_OUT]
                    nc.vector.scalar_tensor_tensor(
                        asl, py[:, 0:D_OUT],
                        sw_all[:, bt * 128 + l0:bt * 128 + l0 + 1],
                        asl, op0=ALU.mult, op1=ALU.add)
                    tmp = tpool.tile([128, D_OUT], F32, tag="tmp")
                    nc.scalar.activation(
                        tmp[:, :], py[:, D_OUT:2 * D_OUT], AF.Copy,
                        scale=sw_all[:, bt * 128 + l0 + 1:
                                     bt * 128 + l0 + 2])
                    nc.gpsimd.tensor_tensor(asl, tmp[:, :], asl,
                                            op=ALU.add)

            # ======== final scale and store ==============================
            for bt in range(N_BT):
                ot = opool.tile([128, D_OUT], F32, tag="o")
                nc.scalar.activation(
                    ot[:, :], acc[:, bt * D_OUT:(bt + 1) * D_OUT],
                    AF.Copy, scale=scol[:, 0:1])
                nc.sync.dma_start(out_c[bt * 128:(bt + 1) * 128, :],
                                  ot[:, :])

    nc.compile()
    return nc


def _prep_inputs(x, W_pre, b_pre, right_w, W_leaf, b_leaf, route_idx,
                 route_side):
    x = np.asarray(x, np.float32)
    W_pre = np.asarray(W_pre, np.float32)
    b_pre = np.asarray(b_pre, np.float32)
    right_w = np.asarray(right_w, np.float32)
    W_leaf = np.asarray(W_leaf, np.float32)
    b_leaf = np.asarray(b_leaf, np.float32)
    route_idx = np.asarray(route_idx)
    route_side = np.asarray(route_side)

    xT = np.ascontiguousarray(x.T).astype(ml_dtypes.bfloat16)  # [512, 8192]
    wpre = np.ascontiguousarray(W_pre.T).astype(ml_dtypes.bfloat16)
    bpre = np.ascontiguousarray(b_pre.reshape(D1, 1))
    nw = right_w / np.maximum(
        np.linalg.norm(right_w, axis=1, keepdims=True), 1e-12)
    nwt = np.ascontiguousarray(nw.T).astype(ml_dtypes.bfloat16)  # [513, 127]

    M = np.zeros((256, 128), np.float32)
    n_steps = route_idx.shape[1]
    for leaf in range(NUM_LEAVES):
        for d in range(n_steps):
            node = int(route_idx[leaf, d])
            side = int(route_side[leaf, d])
            M[node + (128 if side else 0), leaf] += 1.0

    wleaf = np.ascontiguousarray(W_leaf[:, :D_IN].T).astype(
        ml_dtypes.bfloat16)                             # [512, 32768]
    wbcat = np.ascontiguousarray(np.concatenate(
        [W_leaf[:, D_IN].reshape(NUM_LEAVES, D_OUT),
         b_leaf.reshape(NUM_LEAVES, D_OUT)],
        axis=1)).astype(ml_dtypes.bfloat16)             # [128, 512]

    shared = {"wpre": wpre, "bpre": bpre, "nwt": nwt, "mroute": M,
              "wleaf": wleaf, "wbcat": wbcat}
    in_maps = []
    for c in range(N_CORES):
        m = dict(shared)
        m["xT_c"] = np.ascontiguousarray(
            xT[:, c * B_CORE:(c + 1) * B_CORE])
        in_maps.append(m)
    return in_maps


def kernel(x, W_pre, b_pre, right_w, W_leaf, b_leaf, route_idx, route_side):
    global _COMPILED
    if _COMPILED is None:
        _COMPILED = _build_program()
    nc = _COMPILED
    in_maps = _prep_inputs(x, W_pre, b_pre, right_w, W_leaf, b_leaf,
                           route_idx, route_side)
    res = run_bass_kernel_spmd(nc, in_maps, core_ids=list(range(N_CORES)))
    out = np.concatenate([res.results[c]["out_c"] for c in range(N_CORES)],
                         axis=0)
    return out.astype(np.float32)
